# revision 25
# baseline (speedup 1.0000x reference)
"""Trainium2 Bass kernel for nn_DSQGAttentionQW (sparse offset attention).

Sharding: fully head-tensor-parallel (8 heads -> 8 cores). Each core computes
its head's attention over the full sequence plus that head's row-slice of the
output projection, y_h = (attn_h * gate_h / denom_h) @ W_out[64h:64h+64, :].
The host sums the 8 partial projections and adds b_out — no on-device
collective at all, so cross-core launch skew never serializes.
"""
import math
import numpy as np

import concourse.bacc as bacc
import concourse.bass as bass
import concourse.tile as tile
import concourse.mybir as mybir
import concourse.masks as masks
from concourse.bass_utils import run_bass_kernel_spmd

# ---- problem constants (must match reference.py) ----
_DENSE_LOCAL_W = 32
_DYADIC = [48, 64, 96, 128, 192, 256, 384, 512, 768, 1024, 1536, 2048, 3072, 4096]
OFFSETS = np.array(
    sorted(set(range(0, _DENSE_LOCAL_W + 1)) | set(_DYADIC)), dtype=np.int32
)  # [47]
NUM_OFFSETS = len(OFFSETS)
H = 8
_LOG_MAX = math.log(1.0 + 4096.0)
_HEAD_OMEGAS = [0.0, 0.0, 1 * math.pi / _LOG_MAX, 1 * math.pi / _LOG_MAX,
                4 * math.pi / _LOG_MAX, 4 * math.pi / _LOG_MAX,
                6 * math.pi / _LOG_MAX, 6 * math.pi / _LOG_MAX]
_log_d = np.log(1.0 + OFFSETS.astype(np.float64))
DISP_COS_KERNEL = np.zeros((NUM_OFFSETS, H), dtype=np.float32)
for _h, _om in enumerate(_HEAD_OMEGAS):
    if _om > 0.0:
        DISP_COS_KERNEL[:, _h] = np.cos(_om * _log_d)

B, N, D = 1, 2048, 512
HD = D // H
NC = 8
NT = N // 128           # 16 q-tiles of 128
# Effective k-tile depths m (delta in (128(m-1), 128m]) that can be causal for
# N=2048: depths 16/24/32 (delta >= 2048) are never valid.
R_DEPTHS = [0, 1, 2, 3, 4, 6, 8, 12]
NR = len(R_DEPTHS)
RING = 12               # live out2 column-block window (max depth 12 handled
                        # by writing the recycled slot after its tail)

FP = mybir.dt.float32
FR = mybir.dt.float32r
F16 = mybir.dt.float16

_cache = {}


def _build_masks(eff_pb_h: np.ndarray) -> np.ndarray:
    """maskW[ri, kp, i] = exp(eff_pb[offset_idx(delta)]) if delta valid else 0,
    with delta = i - kp + 128*m for depth m = R_DEPTHS[ri]."""
    off_idx = {int(d): i for i, d in enumerate(OFFSETS)}
    kp = np.arange(128)[None, :, None]
    i = np.arange(128)[None, None, :]
    m = np.array(R_DEPTHS)[:, None, None]
    delta = i - kp + 128 * m  # [NR, 128, 128]
    w = np.zeros((NR, 128, 128), dtype=np.float32)
    for d, oi in off_idx.items():
        sel = delta == d
        if sel.any():
            w[sel] = math.exp(float(eff_pb_h[oi]))
    return w


def _build_module():
    nc = bacc.Bacc("TRN2", target_bir_lowering=False, debug=False, num_devices=NC)

    xT = nc.dram_tensor("xT", [4, 128, 4, 512], F16, kind="ExternalInput").ap()
    wA = nc.dram_tensor("wA", [128, 4, 128], F16, kind="ExternalInput").ap()   # [Wq|Wk]
    wB = nc.dram_tensor("wB", [128, 4, 128], F16, kind="ExternalInput").ap()   # [Wv|Wg]
    bA = nc.dram_tensor("bA", [128], FP, kind="ExternalInput").ap()
    bB = nc.dram_tensor("bB", [128], FP, kind="ExternalInput").ap()
    maskW = nc.dram_tensor("maskW", [128, NR, 128], F16, kind="ExternalInput").ap()
    woutH = nc.dram_tensor("woutH", [HD, D], F16, kind="ExternalInput").ap()
    yout = nc.dram_tensor("y", [N, D], F16, kind="ExternalOutput").ap()
    denD = nc.dram_tensor("den", [1, N], F16, kind="ExternalOutput").ap()

    with tile.TileContext(nc) as tc:
        with (
            tc.tile_pool(name="singles", bufs=1) as S,
            tc.tile_pool(name="work", bufs=6) as W,
            tc.tile_pool(name="pk", bufs=4) as PK,
            tc.tile_pool(name="ps", bufs=1, space="PSUM") as PS,
            tc.tile_pool(name="ps3", bufs=3, space="PSUM") as PS3,
            tc.tile_pool(name="pso", bufs=1, space="PSUM") as PSO,
        ):
            # ---------- PE warm-up (HAM) during initial DMA window ----------
            wconst = S.tile([128, 512], F16)
            nc.vector.memset(wconst[:], 1.0)
            psW = PS3.tile([128, 512], FP, tag="s")
            for _ in range(9):
                nc.tensor.matmul(psW[:], wconst[:, 0:128], wconst[:],
                                 start=True, stop=True)

            # out2r: [65, N] PSUM accumulator (4 banks). Zero-init via K=1
            # start=True matmuls (start clears has_written for the WHOLE bank,
            # so it must happen once per bank up front; they also double as PE
            # warm-up); all attention matmuls then accumulate start=False.
            out2r_pre = PSO.tile([HD + 1, N], FP, name="out2r")
            zrow = S.tile([1, 512], F16)
            nc.vector.memset(zrow[:], 0.0)
            z65 = S.tile([1, HD + 1], F16)
            nc.vector.memset(z65[:], 0.0)
            for c in range(4):
                nc.tensor.matmul(out2r_pre[:, 512 * c:512 * (c + 1)], z65[:],
                                 zrow[:], start=True, stop=False)

            # ---------- constants / loads ----------
            ident = S.tile([128, 128], F16)
            masks.make_identity(nc, ident[:])

            wAs = S.tile([128, 4, 128], F16)
            nc.sync.dma_start(out=wAs[:], in_=wA)
            wBs = S.tile([128, 4, 128], F16)
            nc.sync.dma_start(out=wBs[:], in_=wB)
            bAs = S.tile([128, 1], FP)
            nc.sync.dma_start(out=bAs[:], in_=bA[:, None])
            bBs = S.tile([128, 1], FP)
            nc.sync.dma_start(out=bBs[:], in_=bB[:, None])

            xs = S.tile([128, 4, 4, 512], F16)   # [p, nch, ct, n]
            nc.sync.dma_start(out=xs[:, 0, 0:2], in_=xT[0, :, 0:2])
            nc.sync.dma_start(out=xs[:, 0, 2:4], in_=xT[0, :, 2:4])
            for nch in range(1, 4):
                nc.sync.dma_start(out=xs[:, nch], in_=xT[nch])

            mws = S.tile([128, NR, 128], F16)
            nc.sync.dma_start(out=mws[:], in_=maskW)
            wos = S.tile([HD, D], F16)
            nc.sync.dma_start(out=wos[:], in_=woutH[:])

            # ---------- MM-A: qkA / qkB / vT / gT (N=512 chunks) ----------
            # qkA = [q; k], qkB = [k; q] on partitions 0:64 / 64:128 so score
            # matmuls can issue pairwise to distinct PE row-groups (K=64
            # concurrency via tile_position auto-derived from base partition).
            qkA = S.tile([128, N], F16)     # q pre-scaled by 1/sqrt(HD)
            qkB = S.tile([128, N], F16)
            vT = S.tile([64, N], F16)       # if_gain folded
            gT = S.tile([HD + 1, N], FP)   # sigmoid gate; row 64 = 1.0 (denom)
            nc.vector.memset(gT[HD:HD + 1, :], 1.0)
            Vn = S.tile([128, NT, HD + 1], F16)
            nc.vector.memset(Vn[:, :, HD:HD + 1], 1.0)

            def emit_transpose(t):
                psT = PS3.tile([128, 64], F16, tag="s")
                nc.tensor.transpose(psT[:], vT[:, 128 * t:128 * (t + 1)],
                                    ident[0:64, 0:64])
                nc.vector.tensor_copy(Vn[:, t, 0:HD], psT[:])

            for nch in range(4):
                nsl = slice(512 * nch, 512 * (nch + 1))
                psA = PS.tile([128, 512], FP, tag="mma")
                psB = PS3.tile([128, 512], FP, tag="s")
                for ct in range(4):
                    nc.tensor.matmul(psA[:], wAs[:, ct, :], xs[:, nch, ct, :],
                                     start=(ct == 0), stop=(ct == 3))
                for ct in range(4):
                    nc.tensor.matmul(psB[:], wBs[:, ct, :], xs[:, nch, ct, :],
                                     start=(ct == 0), stop=(ct == 3))
                # biased copies: q,k on DVE; v identity / gate sigmoid on ACT
                nc.vector.tensor_scalar_add(qkA[:, nsl], psA[:], bAs[:])
                nc.vector.tensor_copy(qkB[0:64, nsl], qkA[64:128, nsl])
                nc.vector.tensor_copy(qkB[64:128, nsl], qkA[0:64, nsl])
                nc.scalar.activation(vT[:, nsl], psB[0:64, :],
                                     mybir.ActivationFunctionType.Identity,
                                     bias=bBs[0:64], scale=1.0)
                nc.scalar.activation(gT[0:HD, nsl], psB[64:128, :],
                                     mybir.ActivationFunctionType.Sigmoid,
                                     bias=bBs[64:128], scale=1.0)
                if nch >= 1:
                    for t in range(4 * (nch - 1), 4 * nch):
                        emit_transpose(t)

            for t in range(12, 16):
                emit_transpose(t)

            # filler matmuls: bridge the PE-idle gap between MM-A and the
            # attention loop so the HAM clock-gate never re-throttles (a >3.4us
            # PE idle drops the PE from 2.4 to 1.2 GHz and attention's micro-
            # gaps never re-qualify for un-throttle).  Target reuses the PS
            # bank (psA is dead once its epilogue reads finish).
            wfill = PS.tile([128, 512], FP, tag="mma")
            for _ in range(14):
                nc.tensor.matmul(wfill[:], wconst[:, 0:128], wconst[:],
                                 start=True, stop=True)

            # ---------- attention (k-tile-major, fp16) ----------
            out2r = out2r_pre

            zX = S.tile([HD + 1, N], F16)  # gated attn out^T + denom row
            ysbs = [None] * NT

            Pks = [None] * NT

            def emit_mm1(tk):
                ksl = slice(128 * tk, 128 * (tk + 1))
                # group order: m12 first — with 4 psS allocations rotating over
                # 3 PSUM bufs, the 4th group's buffer WAR lands on the same
                # iteration's first group; making that group the tiny m12 (its
                # EXP finishes early) keeps the PE from stalling.  Bases
                # alternate 0/64 by position so adjacent K=64 score matmuls
                # pair onto distinct PE row-groups.
                groups = []
                if tk + 12 < NT:
                    groups.append(([12], tk + 12, 1))
                nq_near = min(3, NT - tk)
                groups.append(([0, 1, 2][:nq_near], tk, nq_near))
                nq34 = max(0, min(2, NT - tk - 3))
                if nq34:
                    groups.append(([3, 4][:nq34], tk + 3, nq34))
                if tk + 8 < NT:
                    groups.append(([6, None, 8], tk + 6, 3))
                elif tk + 6 < NT:
                    groups.append(([6], tk + 6, 1))
                Pk = PK.tile([128, NR, 128], F16, tag="Pk")
                Pks[tk] = Pk
                psSs = []
                for gi, (ms, q_lo, nq) in enumerate(groups):
                    qsl = slice(128 * q_lo, 128 * (q_lo + nq))
                    psS = PS3.tile([128, 384], FP, tag="s")
                    psSs.append(psS)
                    if gi % 2 == 0:
                        nc.tensor.matmul(psS[:, 0:128 * nq], qkB[0:64, ksl],
                                         qkA[0:64, qsl], start=True, stop=True)
                    else:
                        nc.tensor.matmul(psS[:, 0:128 * nq], qkA[64:128, ksl],
                                         qkB[64:128, qsl], start=True, stop=True)
                for gi, (ms, q_lo, nq) in enumerate(groups):
                    psS = psSs[gi]
                    expS = W.tile([128, 384], F16, tag="expS")
                    nc.scalar.activation(expS[:, 0:128 * nq], psS[:, 0:128 * nq],
                                         mybir.ActivationFunctionType.Exp)
                    ris = [R_DEPTHS.index(m) for m in ms if m is not None]
                    if ms == [6, None, 8]:
                        b = expS[:]
                        srcap = bass.AP(tensor=b.tensor, offset=b.offset,
                                        ap=[b.ap[0], [256, 2], [1, 128]])
                        nc.gpsimd.tensor_mul(Pk[:, ris[0]:ris[0] + 2, :],
                                             srcap, mws[:, ris[0]:ris[0] + 2, :])
                    elif ris[0] >= 5:
                        nc.gpsimd.tensor_mul(
                            Pk[:, ris[0]:ris[0] + len(ris), :],
                            expS[:, 0:128 * len(ris)],
                            mws[:, ris[0]:ris[0] + len(ris), :])
                    else:
                        nc.vector.tensor_mul(
                            Pk[:, ris[0]:ris[0] + len(ris), :],
                            expS[:, 0:128 * len(ris)],
                            mws[:, ris[0]:ris[0] + len(ris), :])

            def emit_mm2(tk):
                Pk = Pks[tk]
                nq1 = min(4, NT - tk)
                nc.tensor.matmul(
                    out2r[:, 128 * tk:128 * (tk + nq1)],
                    Vn[:, tk, :], Pk[:, 0:nq1, :],
                    start=False, stop=(tk == NT - 1), skip_group_check=True)
                nfar = sum(1 for m in (4, 6, 8) if tk + m < NT)
                if nfar:
                    b = out2r[:]
                    outap = bass.AP(tensor=b.tensor,
                                    offset=b.offset + 128 * (tk + 4),
                                    ap=[b.ap[0], [256, nfar], [1, 128]])
                    nc.tensor.matmul(outap, Vn[:, tk, :],
                                     Pk[:, 4:4 + nfar, :],
                                     start=False, stop=False,
                                     skip_group_check=True)
                if tk + 12 < NT:
                    nc.tensor.matmul(
                        out2r[:, 128 * (tk + 12):128 * (tk + 13)],
                        Vn[:, tk, :], Pk[:, 7, :],
                        start=False, stop=False, skip_group_check=True)

            def emit_tail(tq):
                """Column block tq of out2r is final: gate-mul, project with
                the W_out row-slice, and stream unnormalized partial y rows
                plus the denominator row out (host divides and sums)."""
                csl = slice(128 * tq, 128 * (tq + 1))
                nc.vector.tensor_mul(zX[:, csl], out2r[:, csl], gT[:, csl])
                if tq >= 12 and tq % 2 == 1:
                    psY = PS3.tile([128, D], FP, tag="s")
                else:
                    psY = PS.tile([128, D], FP, tag="mma")
                nc.tensor.matmul(psY[:], zX[0:HD, csl], wos[:],
                                 start=True, stop=True)
                if tq % 2 == 0:
                    ysbs[tq] = W.tile([128, 2, D], F16, tag="ysb", name=f"ysb2_{tq}")
                ysb2 = ysbs[tq - tq % 2]
                if tq >= 10 and tq % 2 == 1:
                    nc.scalar.copy(ysb2[:, tq % 2, :], psY[:])
                else:
                    nc.vector.tensor_copy(ysb2[:, tq % 2, :], psY[:])
                if tq % 2 == 1:
                    outr = yout[128 * (tq - 1):128 * (tq + 1), :]
                    nc.sync.dma_start(
                        out=outr.rearrange("(j p) d -> p j d", p=128),
                        in_=ysb2[:])

            for tk in range(NT):
                emit_mm1(tk)
                if tk >= 1:
                    emit_mm2(tk - 1)
                    emit_tail(tk - 1)
            emit_mm2(NT - 1)
            emit_tail(NT - 1)
            nc.sync.dma_start(out=denD[:], in_=zX[HD:HD + 1, :])

    nc.compile()
    return nc


def _prep_inputs(x, W_qkv, b_qkv, W_out, b_out, W_gate, b_gate,
                 pos_bias, scale_embed, if_gain, disp_amp):
    assert not np.any(np.asarray(scale_embed)), \
        "kernel fast path requires scale_embed == 0"
    xTn = np.ascontiguousarray(np.asarray(x)[0].T.astype(np.float32))  # [D, N]
    W_qkv = np.asarray(W_qkv, dtype=np.float32)
    b_qkv = np.asarray(b_qkv, dtype=np.float32)
    W_gate = np.asarray(W_gate, dtype=np.float32)
    b_gate = np.asarray(b_gate, dtype=np.float32)
    W_out = np.asarray(W_out, dtype=np.float32)
    pos_bias = np.asarray(pos_bias, dtype=np.float32)
    if_gain = np.asarray(if_gain, dtype=np.float32)
    disp_amp = np.asarray(disp_amp, dtype=np.float32)

    scl = 1.0 / math.sqrt(HD)
    xT16 = np.ascontiguousarray(
        xTn.astype(np.float16).reshape(4, 128, 4, 512).transpose(2, 1, 0, 3))

    in_maps = []
    for h in range(NC):
        qs = slice(HD * h, HD * (h + 1))
        ks = slice(D + HD * h, D + HD * (h + 1))
        vs = slice(2 * D + HD * h, 2 * D + HD * (h + 1))
        wq = W_qkv[:, qs] * scl
        wk = W_qkv[:, ks]
        wv = W_qkv[:, vs] * if_gain[h]
        wg = W_gate[:, qs]
        bq = b_qkv[qs] * scl
        bk = b_qkv[ks]
        bv = b_qkv[vs] * if_gain[h]
        bg = b_gate[qs]
        eff_pb_h = pos_bias[:, h] + DISP_COS_KERNEL[:, h] * disp_amp[h]
        in_maps.append({
            "xT": xT16,
            "wA": np.ascontiguousarray(
                np.concatenate([wq, wk], axis=1).reshape(4, 128, 128)
                .transpose(1, 0, 2)).astype(np.float16),
            "wB": np.ascontiguousarray(
                np.concatenate([wv, wg], axis=1).reshape(4, 128, 128)
                .transpose(1, 0, 2)).astype(np.float16),
            "bA": np.ascontiguousarray(np.concatenate([bq, bk])),
            "bB": np.ascontiguousarray(np.concatenate([bv, bg])),
            "maskW": np.ascontiguousarray(
                _build_masks(eff_pb_h).transpose(1, 0, 2)).astype(np.float16),
            "woutH": np.ascontiguousarray(W_out[qs, :]).astype(np.float16),
        })
    return in_maps


def kernel(**inputs) -> np.ndarray:
    if "nc" not in _cache:
        _cache["nc"] = _build_module()
    nc = _cache["nc"]
    in_maps = _prep_inputs(**inputs)
    res = run_bass_kernel_spmd(nc, in_maps, core_ids=list(range(NC)))
    y = np.zeros((N, D), dtype=np.float32)
    for c in range(NC):
        den = res.results[c]["den"].astype(np.float32).reshape(N, 1)
        y += res.results[c]["y"].astype(np.float32) / den
    y += np.asarray(inputs["b_out"], dtype=np.float32)[None, :]
    return y.reshape(B, N, D)
